# revision 52
# baseline (speedup 1.0000x reference)
"""Trainium2 Bass kernel for nn_DiffForest (soft decision forest forward).

Math: per tree t, z = x @ w_d[t]; p = sigmoid(z); leaf path probs are products
of 8 factors p/(1-p) down a depth-8 tree; output = sum_t leaf_prob @ softmax(w_l[t]) / 10.

Kernel formulation (all on device except small weight prep):
  - The 512 "leaves" come in identical pairs -> fold to 256 paths; fold the
    pair-sum + 1/n_trees into the leaf weight matrix w2 (host, exact).
  - Depth-7 split: path products go to log space only for the first 7 levels:
        C7[q7] = sum_{n<7} softplus(-z_node) + sum_{branch=1} z_node
    which is a matmul with a constant 0/1 matrix S7 [256, 128] (contraction
    [sp(-z); z] over the 127 internal nodes, 2 accumulating matmuls).
    The 8th level is handled elementwise:
        leaf_prob[2*q7]   = exp(-C7[q7]) * sigmoid(z_leaf[q7])
        leaf_prob[2*q7+1] = exp(-C7[q7]) * (1 - sigmoid(z_leaf[q7]))
    with sigmoid(z) = exp(-softplus(-z)) reusing the same Exp/Ln tables.
    This cuts stage-2 PE time 4x vs a full [512, 256] S-matmul.
  - Host permutes w_d columns so cols 0..126 are the internal nodes and cols
    128..255 are the level-7 (leaf-split) nodes, partition-major for
    contiguous DMA. w2 rows are split even/odd with the odd block negated so
    Q1 = (sigmoid - 1) * P7 feeds the leaf matmul directly.
  - Three matmul stages (decision bf16, S7 bf16, leaf bf16), contraction on
    partitions throughout; no on-device transposes.
  - Sharding: data-parallel over batch; each of the 8 cores takes 2048 rows
    of x, weights replicated, no collectives.
  - DMA issue spread across queues: x on Sync, w_d on Scalar (both HWDGE),
    w2/smat/out on GpSimd, so the startup critical path isn't serialized
    behind one queue.
"""

import numpy as np
import ml_dtypes

import concourse.bacc as bacc
import concourse.mybir as mybir
import concourse.tile as tile
from concourse.tile import add_dep_helper
from concourse.bass_utils import run_bass_kernel_spmd

N_CORES = 8
BATCH = 16384
B_LOC = BATCH // N_CORES        # 2048 rows per core
IN_DIM = 2048
N_TREES = 10
ND_PAD = 256                    # decision node columns after permute+pad
CLASSES = 1000
CHUNK = 512                     # batch columns processed per chunk
KI = IN_DIM // 128              # 16 contraction tiles for the decision matmul

BF16 = mybir.dt.bfloat16
F32 = mybir.dt.float32
F16 = mybir.dt.float16
AF = mybir.ActivationFunctionType
ALU = mybir.AluOpType

_CACHE = {}


def _steered_act_tables(orig_fn):
    """Steer Exp and Ln to the combined natural_log_exp_and_others ACT
    table set: this kernel's entire scalar chain then runs off ONE table
    load (zero table swaps; the greedy per-function chooser would
    otherwise alternate exp_and_others / natural_log every block)."""

    def patched(arch):
        out = {}
        for name, s in orig_fn(arch).items():
            s2 = set(s)
            if name != "natural_log_exp_and_others":
                s2.discard(AF.Exp)
                s2.discard(AF.Ln)
            out[name] = s2
        return out

    return patched


def _build(b_loc=B_LOC, n_trees=N_TREES):
    n_chunks = b_loc // CHUNK
    nc = bacc.Bacc("TRN2", target_bir_lowering=False)
    xt = nc.dram_tensor(
        "xt", (b_loc // CHUNK, 128, KI, CHUNK), BF16, kind="ExternalInput"
    )
    wd = nc.dram_tensor("wd", (n_trees, 128, KI, ND_PAD), BF16, kind="ExternalInput")
    smat = nc.dram_tensor("smat", (2, 128, 128), BF16, kind="ExternalInput")
    w2 = nc.dram_tensor("w2", (n_trees, 2, 128, CLASSES), BF16, kind="ExternalInput")
    out = nc.dram_tensor("out", (b_loc, CLASSES), F32, kind="ExternalOutput")

    with tile.TileContext(nc) as tc:
        with (
            tc.tile_pool(name="const", bufs=1) as constp,
            tc.tile_pool(name="sb", bufs=2) as sb,
            tc.tile_pool(name="wdp", bufs=3) as wdp,
            tc.tile_pool(name="ep", bufs=6) as ep,
            tc.tile_pool(name="gp", bufs=6) as gp,
            tc.tile_pool(name="s1p", bufs=6) as s1p,
            tc.tile_pool(name="sgp", bufs=4) as sgp,
            tc.tile_pool(name="qp", bufs=2) as qp,
            tc.tile_pool(name="outp", bufs=2) as outp,
            tc.tile_pool(name="pz", bufs=2, space="PSUM") as pzp,
            tc.tile_pool(name="pc", bufs=2, space="PSUM") as pcp,
            tc.tile_pool(name="po", bufs=2, space="PSUM") as pop,
        ):
            smat_sb = constp.tile([128, 2, 128], BF16)
            w2_sb = constp.tile([128, n_trees, 2, CLASSES], BF16)

            # PE p-state warmup: the PE clock ramps to full speed on
            # sustained duty cycle (early matmuls otherwise run at 2x
            # cycle time). Fill the whole idle window between the start
            # barrier and the first operand's DMA arrival (~2.6us) with
            # back-to-back dummy matmuls so the ramp is underway before
            # real work begins.
            warm = constp.tile([128, 128], BF16)
            nc.vector.memset(warm[:, :], 0.0)
            pw = pzp.tile([128, CHUNK], F32, tag="psz")
            for _ in range(24):
                nc.tensor.matmul(
                    pw[:, 0:128], warm[:, :], warm[:, :],
                    start=True, stop=True,
                )

            GROUP = 5
            first_mm = [None]
            tree_mm = {}
            partial_po = {}
            started = False

            def emit_mm2(ci, Qt, last_chunk=False):
                c0 = ci * CHUNK
                n_acc = n_trees * 2
                for s in range(CHUNK // 128):
                    osb = outp.tile([128, CLASSES], F32, tag="osb")
                    if last_chunk and s == CHUNK // 128 - 1:
                        # final block: accumulate per column-half so the
                        # first half's copy+store overlaps the second
                        # half's matmuls (shorter tail)
                        po = pop.tile([128, 1024], F32, tag="po")
                        for half in range(2):
                            cl = half * 500
                            pl = half * 512
                            i = 0
                            for t in range(n_trees):
                                for lt in range(2):
                                    nc.tensor.matmul(
                                        po[:, pl : pl + 500],
                                        Qt[:, t, lt, s * 128 : (s + 1) * 128],
                                        w2_sb[:, t, lt, cl : cl + 500],
                                        start=(i == 0), stop=(i == n_acc - 1),
                                    )
                                    i += 1
                            rows = out[c0 + s * 128 : c0 + (s + 1) * 128, :]
                            if half == 0:
                                nc.vector.tensor_copy(
                                    osb[:, 0:500], po[:, 0:500]
                                )
                                nc.sync.dma_start(rows[:, 0:500], osb[:, 0:500])
                            else:
                                # split the very last copy across vector and
                                # scalar, and the store in two so the final
                                # transfer is small
                                nc.vector.tensor_copy(
                                    osb[:, 500:750], po[:, 512:762]
                                )
                                nc.scalar.activation(
                                    osb[:, 750:1000], po[:, 762:1012],
                                    AF.Identity,
                                )
                                nc.sync.dma_start(
                                    rows[:, 500:750], osb[:, 500:750]
                                )
                                nc.sync.dma_start(
                                    rows[:, 750:1000], osb[:, 750:1000]
                                )
                        continue
                    if last_chunk and s in partial_po:
                        # trees 0..GROUP-1 already accumulated during the
                        # second group's scalar chain
                        po = partial_po[s]
                        i = GROUP * 2
                        trees = range(GROUP, n_trees)
                        skip = True
                    else:
                        po = pop.tile([128, 1024], F32, tag="po")
                        i = 0
                        trees = range(n_trees)
                        skip = False
                    for t in trees:
                        for lt in range(2):
                            first = i == 0
                            last = i == n_acc - 1
                            lhsT = Qt[:, t, lt, s * 128 : (s + 1) * 128]
                            nc.tensor.matmul(
                                po[:, 0:500], lhsT, w2_sb[:, t, lt, 0:500],
                                start=first, stop=last, skip_group_check=skip,
                            )
                            nc.tensor.matmul(
                                po[:, 512:1012], lhsT, w2_sb[:, t, lt, 500:1000],
                                start=first, stop=last, skip_group_check=skip,
                            )
                            i += 1
                    nc.vector.tensor_copy(osb[:, 0:500], po[:, 0:500])
                    nc.vector.tensor_copy(osb[:, 500:1000], po[:, 512:1012])
                    nc.sync.dma_start(
                        out[c0 + s * 128 : c0 + (s + 1) * 128, :], osb[:, :]
                    )

            for ci in range(n_chunks):
                c0 = ci * CHUNK
                xts = []
                for h in range(2):
                    xp = sb.tile([128, 8, CHUNK], BF16, tag=f"xt{h}")
                    if ci == 0 and h == 0:
                        # sliver the first piece so the first matmuls only
                        # wait on small just-in-time transfers
                        for kl, kr in ((0, 1), (1, 2), (2, 3), (3, 4), (4, 8)):
                            nc.sync.dma_start(
                                xp[:, kl:kr, :], xt[0, :, kl:kr, :]
                            )
                    elif ci == 0:
                        pass  # emitted inside tree 0, gated on the first mm
                    else:
                        xdma = nc.sync.dma_start(
                            xp[:, :, :], xt[ci, :, 8 * h : 8 * (h + 1), :]
                        )
                        # pace: don't let future-chunk x transfers steal HBM
                        # bandwidth from the current chunk's weight stream
                        gate = tree_mm[(ci - 1, 8 if ci == 1 else 1)]
                        add_dep_helper(
                            xdma.ins, gate.ins, sync=True,
                            reason="pace chunk x loads",
                        )
                    xts.append(xp)
                Qt = qp.tile([128, n_trees, 2, CHUNK], BF16, tag="Q")
                for t0 in range(0, n_trees, GROUP):
                    group = list(range(t0, min(t0 + GROUP, n_trees)))
                    gG = {}
                    gS1 = {}
                    for t in group:
                        wd_sb = wdp.tile([128, KI, ND_PAD], BF16, tag="wd")
                        if ci == 0 and t == 0:
                            wd0_k0 = nc.scalar.dma_start(
                                wd_sb[:, 0:1, :], wd[t, :, 0:1, :]
                            )
                            for kl, kr in ((1, 2), (2, 3), (3, 4), (4, 8), (8, 16)):
                                nc.scalar.dma_start(
                                    wd_sb[:, kl:kr, :], wd[t, :, kl:kr, :]
                                )
                            # second xt half (k8-15): start after the weight
                            # slivers so the startup-critical transfers get
                            # the HBM bandwidth first
                            for xh, kl in ((0, 8), (1, 12)):
                                xdma = nc.sync.dma_start(
                                    xts[1][:, 4 * xh : 4 * (xh + 1), :],
                                    xt[0, :, kl : kl + 4, :],
                                )
                                add_dep_helper(
                                    xdma.ins, wd0_k0.ins, sync=True,
                                    reason="startup: critical pieces first",
                                )
                        else:
                            wd_dma = nc.scalar.dma_start(wd_sb[:, :, :], wd[t, :, :, :])
                            if ci == 0:
                                # pace the startup weight stream just-in-time
                                # so each 1MB transfer has a clear window
                                if t == 1:
                                    gate = first_mm[0]
                                elif t == 2:
                                    gate = tree_mm[(0, "t0dt1")]
                                else:
                                    gate = tree_mm[(0, t - 2)]
                                add_dep_helper(
                                    wd_dma.ins, gate.ins, sync=True,
                                    reason="startup: critical pieces first",
                                )
                        G = gp.tile([128, 2, CHUNK], BF16, tag="G")
                        E = ep.tile([128, 2, CHUNK], F16, tag="E")
                        S1 = s1p.tile([128, CHUNK], BF16, tag="S1")
                        gG[t] = G
                        gS1[t] = S1
                        for dt_ in range(2):
                            psz = pzp.tile([128, CHUNK], F32, tag="psz")
                            for k in range(KI):
                                lhsT = wd_sb[:, k, dt_ * 128 : (dt_ + 1) * 128]
                                mm = nc.tensor.matmul(
                                    psz[:, :], lhsT, xts[k // 8][:, k % 8, :],
                                    start=(k == 0), stop=(k == KI - 1),
                                )
                                if first_mm[0] is None:
                                    first_mm[0] = mm
                                if k == 0 and dt_ == 0:
                                    tree_mm[(ci, t)] = mm
                                if k == 0 and dt_ == 1 and t == 0:
                                    tree_mm[(ci, "t0dt1")] = mm
                            # softplus(-z) = ln(exp(-z)+1); Exp and Ln live in
                            # the same ACT table set (forced via the table
                            # patch below), so no batching needed — pipeline
                            # per tree for the shortest dependency chains
                            nc.scalar.activation(
                                E[:, dt_, :], psz[:, :], AF.Exp, scale=-1.0
                            )
                            dst = G[:, 0, :] if dt_ == 0 else S1[:, :]
                            nc.scalar.activation(
                                dst, E[:, dt_, :], AF.Ln, bias=1.0
                            )
                            if dt_ == 0:
                                nc.vector.tensor_copy(G[:, 1, :], psz[:, :])
                    if ci == n_chunks - 1 and t0 == GROUP:
                        # last chunk: pre-accumulate the first group's leaf
                        # matmuls for two row-blocks so the PE has work while
                        # the second group's Exp/Ln/Q chain drains
                        for s in range(2):
                            po = pop.tile([128, 1024], F32, tag="po")
                            partial_po[s] = po
                            i = 0
                            for t in range(GROUP):
                                for lt in range(2):
                                    lhsT = Qt[:, t, lt, s * 128 : (s + 1) * 128]
                                    nc.tensor.matmul(
                                        po[:, 0:500], lhsT, w2_sb[:, t, lt, 0:500],
                                        start=(i == 0), stop=False,
                                        skip_group_check=True,
                                    )
                                    nc.tensor.matmul(
                                        po[:, 512:1012], lhsT,
                                        w2_sb[:, t, lt, 500:1000],
                                        start=(i == 0), stop=False,
                                        skip_group_check=True,
                                    )
                                    i += 1
                    if not started:
                        nc.sync.dma_start(
                            smat_sb[:, :, :],
                            smat[:, :, :].rearrange("j p q -> p j q"),
                        )
                        started = True
                    if ci == 0 and t0 == GROUP:
                        # pace the 5MB w2 transfer one tree-piece at a time
                        # behind the startup weight stream, else it starves
                        # the PE of decision weights; it only needs to land
                        # before the first leaf matmul (~80us in)
                        for t in range(n_trees):
                            gate = tree_mm[(0, min(t + 1, n_trees - 1))]
                            wdma = nc.sync.dma_start(
                                w2_sb[:, t, :, :],
                                w2[t, :, :, :].rearrange("l p c -> p l c"),
                            )
                            add_dep_helper(
                                wdma.ins, gate.ins, sync=True,
                                reason="pace w2 load past startup",
                            )
                    for t in group:
                        psC = pcp.tile([128, CHUNK], F32, tag="psC")
                        nc.tensor.matmul(
                            psC[:, :], smat_sb[:, 0, :], gG[t][:, 0, :],
                            start=True, stop=False,
                        )
                        nc.tensor.matmul(
                            psC[:, :], smat_sb[:, 1, :], gG[t][:, 1, :],
                            start=False, stop=True,
                        )
                        PS = sgp.tile([128, 2, CHUNK], BF16, tag="PS")
                        nc.scalar.activation(
                            PS[:, 0, :], psC[:, :], AF.Exp, scale=-1.0
                        )
                        nc.scalar.activation(
                            PS[:, 1, :], gS1[t][:, :], AF.Exp, scale=-1.0
                        )
                        # Q0 = P7 * sig(z_L); Q1m = (sig - 1) * P7  (w2 odd
                        # block is negated on host to absorb the sign)
                        nc.vector.tensor_tensor(
                            Qt[:, t, 0, :], PS[:, 0, :], PS[:, 1, :], ALU.mult
                        )
                        nc.vector.scalar_tensor_tensor(
                            Qt[:, t, 1, :], PS[:, 1, :], 1.0, PS[:, 0, :],
                            ALU.subtract, ALU.mult,
                        )
                emit_mm2(ci, Qt, last_chunk=(ci == n_chunks - 1))
    orig_tables = bacc.get_activation_tables
    bacc.get_activation_tables = _steered_act_tables(orig_tables)
    try:
        nc.compile()
    finally:
        bacc.get_activation_tables = orig_tables
    return nc


def _smat7_np():
    S = np.zeros((2, 128, 128), np.float32)
    q7 = np.arange(128)
    for n in range(7):
        node = (2**n - 1) + (q7 >> (7 - n))
        b = (q7 >> (6 - n)) & 1
        S[0, node, q7] = 1.0
        S[1, node, q7] = b
    return S


def _prep_weights(w_d, w_l, n_trees=N_TREES):
    bf16 = ml_dtypes.bfloat16
    w_l = np.asarray(w_l, dtype=np.float32)
    m = w_l.max(axis=-1, keepdims=True)
    e = np.exp(w_l - m, dtype=np.float32)
    sm = e / e.sum(axis=-1, keepdims=True)
    w2fold = (sm[:, 0::2, :] + sm[:, 1::2, :]) * np.float32(1.0 / n_trees)
    w2p = np.empty((n_trees, 2, 128, CLASSES), np.float32)
    w2p[:, 0] = w2fold[:, 0::2, :]
    w2p[:, 1] = -w2fold[:, 1::2, :]
    # permute decision columns: 0..126 internal nodes, 127 pad,
    # 128..255 level-7 nodes; then partition-major [t, p, k, col]
    wd_cols = np.zeros((n_trees, IN_DIM, ND_PAD), np.float32)
    wd_cols[:, :, 0:127] = w_d[:, :, 0:127]
    wd_cols[:, :, 128:256] = w_d[:, :, 127:255]
    wd_p = np.ascontiguousarray(
        wd_cols.reshape(n_trees, KI, 128, ND_PAD).transpose(0, 2, 1, 3)
    )
    return wd_p.astype(bf16), _smat7_np().astype(bf16), w2p.astype(bf16)


last_bass_results = None


def kernel(x, w_d, w_l):
    global last_bass_results
    x = np.asarray(x)
    wd_bf, S7, w2p = _prep_weights(np.asarray(w_d), np.asarray(w_l))
    x_bf = x.astype(ml_dtypes.bfloat16)
    in_maps = []
    for c in range(N_CORES):
        xc = x_bf[c * B_LOC : (c + 1) * B_LOC, :]
        # [ci, p, k, n] chunk-major so every chunk tile is contiguous
        # per partition (8KB runs) for fast DMA
        xt = np.ascontiguousarray(
            xc.reshape(B_LOC // CHUNK, CHUNK, KI, 128).transpose(0, 3, 2, 1)
        )
        in_maps.append({"xt": xt, "wd": wd_bf, "smat": S7, "w2": w2p})
    if "nc" not in _CACHE:
        _CACHE["nc"] = _build()
    res = run_bass_kernel_spmd(_CACHE["nc"], in_maps, core_ids=list(range(N_CORES)))
    last_bass_results = res
    return np.concatenate([res.results[c]["out"] for c in range(N_CORES)], axis=0)
